# revision 5
# baseline (speedup 1.0000x reference)
"""Trainium2 Bass kernel for nn_Block_15144054685914 (dense transformer block).

Sharding: 8 cores = 2 batch groups (DP) x 4-way tensor parallel.
  core c: batch b = c//4, heads [4*(c%4), 4*(c%4)+4), FFN slice c%4.
One on-device bf16 AllReduce per 512-token chunk (attention residual) within
each 4-core batch group; host sums the 8 partial outputs.

Math tricks (all exact up to float rounding):
  - alpha-softmax K/V basis mixing and the ve head-mix are folded on the host
    into Wk_eff/Wv_eff and a premixed ve input (exact, fp32 on host)
  - rmsnorm(x) scale cancels for Q/K (rmsnorm(rope(c*v)) == rmsnorm(rope(v)))
  - rmsnorm scale for the MLP folds into a per-row post-scale
  - softmax 1/sum folds into a post-PV column scale; row sums via ones-matmul
  - residual carried at 1/4 scale (Wo pre-scaled by 1/4, x fed as x/16) so the
    AllReduce output is 0.25*x1 and doubles as this core's residual partial;
    all scale corrections fold into existing activation scale/bias fields
  - causal masking: off-diagonal key blocks computed unmasked; diagonal key
    blocks only compute the live query range (plus one 128x128 triangle mult)

Per chunk: attention -> AllReduce -> that chunk's MLP, emitted immediately so
the Tile scheduler interleaves MLP matmuls into the next chunk's attention
stalls (PE was 65% busy in the attention phase, 100% in the MLP phase).
"""

import math
import numpy as np
import ml_dtypes

B, E, H, J = 2, 2048, 16, 4
D = 128
GC = 12
FF = 4 * E
NCORES = 8
HL = H // 4            # local heads per core
HDL = HL * D           # 512
JD = J * D             # 512
FL = FF // 4           # 2048 local ffn rows
EPS = float(np.finfo(np.float32).eps)
T_FULL = 2048
CH = 512               # t-chunk for attention + AllReduce
EC = E // 128          # 16
FCT = FL // 128        # 16 f-tiles

bf16n = ml_dtypes.bfloat16
NOAR = False


def _bf(x):
    return np.ascontiguousarray(np.asarray(x, dtype=np.float32)).astype(bf16n)


def shard_inputs(x, ve, cos, sin, Wq, Wk, Wv, Wo, alpha_k, alpha_v, Wg,
                 Wfc, Wmlp, T=T_FULL):
    x = np.asarray(x, np.float32)[:, :T]
    ve = np.asarray(ve, np.float32)[:, :T]
    cosf = np.asarray(cos, np.float32)[0, :T, 0, :]   # (T, 64)
    sinf = np.asarray(sin, np.float32)[0, :T, 0, :]
    Wq = np.asarray(Wq, np.float32)
    Wk = np.asarray(Wk, np.float32)
    Wv = np.asarray(Wv, np.float32)
    Wo = np.asarray(Wo, np.float32)
    Wg = np.asarray(Wg, np.float32)
    Wfc = np.asarray(Wfc, np.float32)
    Wmlp = np.asarray(Wmlp, np.float32)
    alpha_k = np.asarray(alpha_k, np.float32)
    alpha_v = np.asarray(alpha_v, np.float32)

    # host-side alpha softmax + basis mixing (exact fp32)
    def _softmax(a):
        e = np.exp(a - a.max(axis=-1, keepdims=True))
        return e / e.sum(axis=-1, keepdims=True)

    w_k = _softmax(alpha_k)                       # (H, J)
    w_v = _softmax(alpha_v)
    Wk_eff = np.einsum('hj,jde->hde', w_k, Wk.reshape(J, D, E))  # (H,D,E)
    Wv_eff = np.einsum('hj,jde->hde', w_v, Wv.reshape(J, D, E))
    ve_mix = np.einsum('hj,btjd->bthd', w_v, ve.reshape(B, T, J, D))

    tri = (np.arange(128)[:, None] <= np.arange(128)[None, :]).astype(
        np.float32)

    in_maps = []
    for c in range(NCORES):
        b = c // 4
        hg = c % 4
        hs, he = hg * HL, (hg + 1) * HL
        hsl = slice(hg * HDL, (hg + 1) * HDL)      # head-dim slice of E
        fsl = slice(hg * FL, (hg + 1) * FL)        # ffn slice
        m = {
            "xh": _bf(x[b] / 16.0),                            # (T, E)
            "xT": _bf(x[b].T),                                 # (E, T)
            "vem": _bf(ve_mix[b].reshape(T, H * D)[:, hsl]),   # (T, HDL)
            "cos2": _bf(np.concatenate([cosf.T, cosf.T], 0)),  # (128, T)
            "sin2": _bf(np.concatenate([sinf.T, -sinf.T], 0)),  # (128, T)
            "p64": _bf(np.eye(128)[:, list(range(64, 128)) + list(range(64))].T),
            "wqT": _bf(Wq[hsl, :].T),                          # (E, HDL)
            "wkT": _bf(Wk_eff[hs:he].reshape(HDL, E).T),       # (E, HDL)
            "wvT": _bf(Wv_eff[hs:he].reshape(HDL, E).T),       # (E, HDL)
            "woT": _bf(0.25 * Wo.T[hsl, :]),                   # (HDL, E)
            "wfcT": _bf(Wfc.T[:, fsl]),                        # (E, FL)
            "wmlpT": _bf(Wmlp.T[fsl, :]),                      # (FL, E)
            "wgT": _bf(Wg[hs:he, :].T),                        # (GC, HL)
            "tri": _bf(tri),                                   # (128, 128)
            "onec": _bf(np.ones((128, 1))),
            "oner": _bf(np.ones((1, 128))),
        }
        in_maps.append(m)
    return in_maps


def declare_io(nc, T):
    import concourse.mybir as mybir
    bf = mybir.dt.bfloat16
    io = {}

    def inp(name, shape, dt=bf):
        io[name] = nc.dram_tensor(name, list(shape), dt, kind="ExternalInput").ap()

    inp("xh", (T, E)); inp("xT", (E, T)); inp("vem", (T, HDL))
    inp("cos2", (128, T)); inp("sin2", (128, T)); inp("p64", (128, 128))
    inp("wqT", (E, HDL)); inp("wkT", (E, HDL)); inp("wvT", (E, HDL))
    inp("woT", (HDL, E)); inp("wfcT", (E, FL)); inp("wmlpT", (FL, E))
    inp("wgT", (GC, HL)); inp("tri", (128, 128))
    inp("onec", (128, 1)); inp("oner", (1, 128))
    io["out"] = nc.dram_tensor("out", [T, E], bf, kind="ExternalOutput").ap()
    return io


def emit(tc, io, T):
    import concourse.mybir as mybir
    from contextlib import ExitStack

    nc = tc.nc
    bf = mybir.dt.bfloat16
    f32 = mybir.dt.float32
    AF = mybir.ActivationFunctionType
    OP = mybir.AluOpType
    nch = T // CH
    TT = T // 128                  # number of 128-row t-tiles
    qk_ln_scale = 1.0 / (128.0 * 1.44)   # mean over D and the 1.2^2 fold
    inv_sqrt_d = 1.0 / math.sqrt(D)
    LN16 = math.log(16.0)

    with ExitStack() as ctx:
        cpool = ctx.enter_context(tc.tile_pool(name="const", bufs=1))
        big = ctx.enter_context(tc.tile_pool(name="big", bufs=1))
        wk = ctx.enter_context(tc.tile_pool(name="wk", bufs=1))
        colp = ctx.enter_context(tc.tile_pool(name="colp", bufs=1))
        psp = ctx.enter_context(tc.tile_pool(name="psp", bufs=1, space="PSUM"))
        dram = ctx.enter_context(tc.tile_pool(name="dram", bufs=2, space="DRAM"))

        # ---------------- chunk-0 stream prefetch ----------------
        xt_tiles = {}

        def load_xt(ci):
            t = big.tile([128, EC, CH], bf, name=f"xt{ci}", tag="xt", bufs=2)
            nc.sync.dma_start(
                t[:], io["xT"].rearrange("(a p) t -> p a t", p=128)
                [:, :, ci * CH:(ci + 1) * CH])
            xt_tiles[ci] = t

        load_xt(0)

        # ---------------- constants ----------------
        onec = cpool.tile([128, 1], bf)
        nc.sync.dma_start(onec[:], io["onec"][:])
        oner = cpool.tile([1, 128], bf)
        nc.sync.dma_start(oner[:], io["oner"][:])
        tri = cpool.tile([128, 128], bf)
        nc.sync.dma_start(tri[:], io["tri"][:])
        p64 = cpool.tile([128, 128], bf)
        nc.sync.dma_start(p64[:], io["p64"][:])
        wgT = cpool.tile([GC, HL], bf)
        nc.sync.dma_start(wgT[:], io["wgT"][:])
        eps_e = cpool.tile([128, 1], f32)
        nc.vector.memset(eps_e[:], EPS)
        eps_qk = cpool.tile([1, 1], f32)
        nc.vector.memset(eps_qk[:], EPS / 1.44)
        ln16_e = cpool.tile([128, 1], f32)
        nc.vector.memset(ln16_e[:], LN16)

        kT = big.tile([128, HL, T], bf)           # final K, feature-major
        vtile = big.tile([128, TT, HDL], bf)      # final V, token-major

        cin = dram.tile([T, E], bf)
        cout = dram.tile([T, E], bf)

        groups = [[0, 1, 2, 3], [4, 5, 6, 7]]

        def row_stats_sq(x_tt, name):
            """mean of squares per row of a (128, E) bf16 tile -> (128,1) f32."""
            bnt = colp.tile([128, 4, 6], f32, name=f"{name}_bnt", tag="bnt",
                            bufs=2)
            for i in range(4):
                nc.vector.bn_stats(bnt[:, i, :],
                                   x_tt[:, i * 512:(i + 1) * 512])
            agg = colp.tile([128, 2], f32, name=f"{name}_agg", tag="agg",
                            bufs=2)
            nc.vector.bn_aggr(agg[:], bnt[:])
            m2 = colp.tile([128, 1], f32, name=f"{name}_m2", tag="c1", bufs=8)
            nc.vector.tensor_tensor(m2[:], agg[:, 0:1], agg[:, 0:1], op=OP.mult)
            msq = colp.tile([128, 1], f32, name=f"{name}_msq", tag="c1", bufs=8)
            nc.vector.tensor_tensor(msq[:], m2[:], agg[:, 1:2], op=OP.add)
            return msq

        # ======================= chunk loop =======================
        for c in range(nch):
            csl = slice(c * CH, (c + 1) * CH)
            cos2 = wk.tile([128, CH], bf, name=f"cos2_{c}", tag="cs", bufs=2)
            nc.sync.dma_start(cos2[:], io["cos2"][:, csl])
            sin2 = wk.tile([128, CH], bf, name=f"sin2_{c}", tag="cs", bufs=2)
            nc.sync.dma_start(sin2[:], io["sin2"][:, csl])
            xt = xt_tiles.pop(c)

            # ---- per-tile rmsnorm scale s[t] = rsqrt(mean(x^2)+eps) ----
            scols = []
            for tt in range(4):
                rows = slice(c * CH + tt * 128, c * CH + (tt + 1) * 128)
                xq_tt = wk.tile([128, E], bf, name=f"xq{c}_{tt}", tag="xq",
                                bufs=2)
                nc.sync.dma_start(xq_tt[:], io["xh"][rows, :])
                msq = row_stats_sq(xq_tt, f"s{c}_{tt}")
                lnm = colp.tile([128, 1], f32, name=f"lnm{c}_{tt}", tag="c1",
                                bufs=8)
                # mean(x^2) = 256*msq  (xh = x/16)
                nc.scalar.activation(lnm[:], msq[:], AF.Ln, scale=256.0,
                                     bias=eps_e[:])
                scol = colp.tile([128, 1], f32, name=f"scol{c}_{tt}",
                                 tag="scol", bufs=6)
                nc.scalar.activation(scol[:], lnm[:], AF.Exp, scale=-0.5)
                scols.append(scol)

            # ---- V projection (token-major) + gate + assembly ----
            wv_t = []
            for g in range(4):
                t = wk.tile([128, 4, HDL], bf, name=f"wv{c}_{g}", tag="we3",
                            bufs=5)
                nc.sync.dma_start(
                    t[:], io["wvT"].rearrange("(a p) n -> p a n", p=128)
                    [:, 4 * g:4 * g + 4, :])
                wv_t.append(t)
            for tt in range(4):
                tsl = slice(tt * 128, (tt + 1) * 128)
                rows = slice(c * CH + tt * 128, c * CH + (tt + 1) * 128)
                v_ps = psp.tile([128, HDL], f32, name=f"vps{c}_{tt}",
                                tag="ps", bufs=8)
                for e in range(EC):
                    nc.tensor.matmul(v_ps[:], xt[:, e, tsl],
                                     wv_t[e // 4][:, e % 4, :],
                                     start=(e == 0), stop=(e == EC - 1))
                # gate: 3*sigmoid(h[:, :GC] @ WgT)
                g_ps = psp.tile([128, HL], f32, name=f"gps{c}_{tt}", tag="ps",
                                bufs=8)
                nc.tensor.matmul(g_ps[:], xt[0:GC, 0, tsl], wgT[:],
                                 start=True, stop=True)
                zs = colp.tile([128, HL], f32, name=f"zs{c}_{tt}", tag="g4",
                               bufs=3)
                nc.vector.tensor_scalar(zs[:], g_ps[:], scols[tt][:],
                                        None, op0=OP.mult)
                ge = colp.tile([128, HL], f32, name=f"ge{c}_{tt}", tag="g4",
                               bufs=3)
                nc.scalar.activation(ge[:], zs[:], AF.Exp, scale=-1.0)
                gd = colp.tile([128, HL], f32, name=f"gd{c}_{tt}", tag="g4",
                               bufs=3)
                nc.vector.tensor_scalar(gd[:], ge[:], 1.0, None, op0=OP.add)
                gr = colp.tile([128, HL], f32, name=f"gr{c}_{tt}", tag="g4",
                               bufs=3)
                nc.vector.reciprocal(gr[:], gd[:])
                g3 = colp.tile([128, HL], f32, name=f"g3{c}_{tt}", tag="g3",
                               bufs=2)
                nc.vector.tensor_scalar(g3[:], gr[:], 3.0, None, op0=OP.mult)

                vet = wk.tile([128, HDL], bf, name=f"vet{c}_{tt}", tag="vet",
                              bufs=3)
                nc.sync.dma_start(vet[:], io["vem"][rows, :])
                gv = wk.tile([128, HDL], bf, name=f"gv{c}_{tt}", tag="gv",
                             bufs=2)
                for h in range(HL):
                    nc.vector.tensor_scalar(
                        gv[:, h * D:(h + 1) * D], vet[:, h * D:(h + 1) * D],
                        g3[:, h:h + 1], None, op0=OP.mult)
                nc.vector.scalar_tensor_tensor(
                    vtile[:, c * 4 + tt, :], v_ps[:], scols[tt][:],
                    gv[:], op0=OP.mult, op1=OP.add)

            if c + 1 < nch:
                load_xt(c + 1)

            # ---- q/k rope + norm helper (feature-major) ----
            def rope_norm(src_ps, h, kind, dst):
                sb = wk.tile([128, CH], bf, name=f"{kind}sb{c}_{h}", tag="qk",
                             bufs=5)
                nc.scalar.copy(sb[:], src_ps[:])
                sq = wk.tile([128, CH], bf, name=f"{kind}sq{c}_{h}", tag="qk",
                             bufs=5)
                nc.scalar.activation(sq[:], src_ps[:], AF.Square)
                ss_ps = psp.tile([1, CH], f32, name=f"{kind}ss{c}_{h}",
                                 tag="ps", bufs=8)
                nc.tensor.matmul(ss_ps[:], onec[:], sq[:], start=True,
                                 stop=True)
                lnr = colp.tile([1, CH], bf, name=f"{kind}ln{c}_{h}",
                                tag="r512", bufs=2)
                nc.scalar.activation(lnr[:], ss_ps[:], AF.Ln,
                                     scale=qk_ln_scale, bias=eps_qk[:])
                rs2 = colp.tile([1, CH], bf, name=f"{kind}rs{c}_{h}",
                                tag="r512b", bufs=2)
                nc.scalar.activation(rs2[:], lnr[:], AF.Exp, scale=-0.5)
                rb_ps = psp.tile([128, CH], f32, name=f"{kind}rb{c}_{h}",
                                 tag="ps", bufs=8)
                nc.tensor.matmul(rb_ps[:], oner[:], rs2[:], start=True,
                                 stop=True)
                ta = wk.tile([128, CH], bf, name=f"{kind}ta{c}_{h}", tag="qk",
                             bufs=5)
                tb = wk.tile([128, CH], bf, name=f"{kind}tb{c}_{h}", tag="qk",
                             bufs=5)
                ro = wk.tile([128, CH], bf, name=f"{kind}ro{c}_{h}", tag="qk",
                             bufs=5)
                swp_ps = psp.tile([128, CH], f32, name=f"{kind}sw{c}_{h}",
                                  tag="ps", bufs=8)
                nc.tensor.matmul(swp_ps[:], p64[:], sb[:], start=True,
                                 stop=True)
                nc.vector.tensor_tensor(ta[:], sb[:], cos2[:], op=OP.mult)
                nc.vector.tensor_tensor(tb[:], swp_ps[:], sin2[:], op=OP.mult)
                nc.vector.tensor_tensor(ro[:], ta[:], tb[:], op=OP.add)
                nc.vector.tensor_tensor(dst, ro[:], rb_ps[:], op=OP.mult)

            # ---- per-head: q/k projection + rope + attention ----
            yTfs = []
            for h in range(HL):
                wtq = wk.tile([128, EC, 128], bf, name=f"wq{c}_{h}",
                              tag="we3", bufs=5)
                nc.sync.dma_start(
                    wtq[:], io["wqT"].rearrange("(a p) n -> p a n", p=128)
                    [:, :, h * D:(h + 1) * D])
                q_ps = psp.tile([128, CH], f32, name=f"qps{c}_{h}", tag="ps",
                                bufs=8)
                for e in range(EC):
                    nc.tensor.matmul(q_ps[:], wtq[:, e, :], xt[:, e, :],
                                     start=(e == 0), stop=(e == EC - 1))
                qf = wk.tile([128, CH], bf, name=f"qf{c}_{h}", tag="qf",
                             bufs=3)
                rope_norm(q_ps, h, "q", qf[:])

                wtk = wk.tile([128, EC, 128], bf, name=f"wk{c}_{h}",
                              tag="we3", bufs=5)
                nc.sync.dma_start(
                    wtk[:], io["wkT"].rearrange("(a p) n -> p a n", p=128)
                    [:, :, h * D:(h + 1) * D])
                k_ps = psp.tile([128, CH], f32, name=f"kps{c}_{h}", tag="ps",
                                bufs=8)
                for e in range(EC):
                    nc.tensor.matmul(k_ps[:], wtk[:, e, :], xt[:, e, :],
                                     start=(e == 0), stop=(e == EC - 1))
                rope_norm(k_ps, h, "k", kT[:, h, csl])

                # ---- attention for this head ----
                sums_ps = psp.tile([1, CH], f32, name=f"sums{c}_{h}", tag="ps",
                                   bufs=8)
                yT_ps = psp.tile([128, CH], f32, name=f"yT{c}_{h}", tag="ps",
                                 bufs=8)
                # full (unmasked) key blocks from earlier chunks
                for sb_i in range(4 * c):
                    sc_ps = psp.tile([128, CH], f32, name=f"sc{c}_{h}_{sb_i}",
                                     tag="ps", bufs=8)
                    nc.tensor.matmul(sc_ps[:],
                                     kT[:, h, sb_i * 128:(sb_i + 1) * 128],
                                     qf[:], start=True, stop=True)
                    p0 = wk.tile([128, CH], bf, name=f"p0{c}_{h}_{sb_i}",
                                 tag="p", bufs=4)
                    nc.scalar.activation(p0[:], sc_ps[:], AF.Exp,
                                         scale=inv_sqrt_d)
                    nc.tensor.matmul(sums_ps[:], onec[:], p0[:],
                                     start=(sb_i == 0), stop=False)
                    nc.tensor.matmul(yT_ps[:],
                                     vtile[:, sb_i, h * D:(h + 1) * D],
                                     p0[:], start=(sb_i == 0), stop=False)
                # diagonal key blocks: only live query columns
                for m in range(4):
                    sb_i = 4 * c + m
                    W = CH - m * 128
                    first = (c == 0 and m == 0)
                    last = (m == 3)
                    sc_ps = psp.tile([128, CH], f32, name=f"sc{c}_{h}_{sb_i}",
                                     tag="ps", bufs=8)
                    nc.tensor.matmul(sc_ps[:, 0:W],
                                     kT[:, h, sb_i * 128:(sb_i + 1) * 128],
                                     qf[:, m * 128:CH], start=True, stop=True)
                    p0 = wk.tile([128, CH], bf, name=f"p0{c}_{h}_{sb_i}",
                                 tag="p", bufs=4)
                    nc.scalar.activation(p0[:, 0:W], sc_ps[:, 0:W], AF.Exp,
                                         scale=inv_sqrt_d)
                    nc.vector.tensor_tensor(p0[:, 0:128], p0[:, 0:128],
                                            tri[:], op=OP.mult)
                    nc.tensor.matmul(sums_ps[0:1, m * 128:CH], onec[:],
                                     p0[:, 0:W], start=first, stop=last)
                    nc.tensor.matmul(yT_ps[:, m * 128:CH],
                                     vtile[:, sb_i, h * D:(h + 1) * D],
                                     p0[:, 0:W], start=first, stop=last)
                isr = colp.tile([1, CH], bf, name=f"isr{c}_{h}", tag="r512b",
                                bufs=2)
                with nc.allow_low_precision(reason="softmax 1/sum in bf16"):
                    nc.vector.reciprocal(isr[:], sums_ps[:])
                ib_ps = psp.tile([128, CH], f32, name=f"ib{c}_{h}", tag="ps",
                                 bufs=8)
                nc.tensor.matmul(ib_ps[:], oner[:], isr[:], start=True,
                                 stop=True)
                ib = wk.tile([128, CH], bf, name=f"ibs{c}_{h}", tag="p",
                             bufs=4)
                nc.scalar.copy(ib[:], ib_ps[:])
                yTf = wk.tile([128, CH], bf, name=f"yTf{c}_{h}", tag="y",
                              bufs=4)
                nc.vector.tensor_tensor(yTf[:], yT_ps[:], ib[:], op=OP.mult)
                yTfs.append(yTf)

            # ---- Wo partial (pre-scaled 1/4) + x/16, to AR bounce ----
            if c == 0:
                wot = big.tile([128, HL, E], bf, name="wot", tag="wot",
                               bufs=1)
                nc.sync.dma_start(
                    wot[:], io["woT"].rearrange("(a p) n -> p a n", p=128))
            for tt in range(4):
                tsl = slice(tt * 128, (tt + 1) * 128)
                rows = slice(c * CH + tt * 128, c * CH + (tt + 1) * 128)
                xqw = wk.tile([128, E], bf, name=f"xqw{c}_{tt}", tag="xq",
                              bufs=2)
                nc.sync.dma_start(xqw[:], io["xh"][rows, :])
                for ot in range(4):
                    osl = slice(ot * 512, (ot + 1) * 512)
                    wo_ps = psp.tile([128, 512], f32,
                                     name=f"wops{c}_{tt}_{ot}", tag="ps",
                                     bufs=8)
                    for h in range(HL):
                        nc.tensor.matmul(wo_ps[:], yTfs[h][:, tsl],
                                         wot[:, h, osl], start=(h == 0),
                                         stop=(h == HL - 1))
                    aro = wk.tile([128, 512], bf, name=f"aro{c}_{tt}_{ot}",
                                  tag="p", bufs=4)
                    nc.vector.tensor_tensor(aro[:], wo_ps[:],
                                            xqw[:, osl], op=OP.add)
                    nc.sync.dma_start(cin[rows, osl], aro[:])

            # ---- AllReduce this chunk within the batch group ----
            # cout = 0.25 * x1  (residual partial AND MLP input)
            if NOAR:
                nc.sync.dma_start(cout[csl, :], cin[csl, :])
            else:
                nc.gpsimd.collective_compute(
                    "AllReduce", mybir.AluOpType.add, replica_groups=groups,
                    ins=[cin[csl, :].opt()], outs=[cout[csl, :].opt()])

            # =================== MLP for this chunk ===================
            # x1q = 0.25*x1 rows (token-major): stats + residual partial
            x1_tts = []
            s2cols = []
            for tt in range(4):
                rows = slice(c * CH + tt * 128, c * CH + (tt + 1) * 128)
                x1_tt = wk.tile([128, E], bf, name=f"x1{c}_{tt}", tag="mq",
                                bufs=4)
                nc.sync.dma_start(x1_tt[:], cout[rows, :])
                x1_tts.append(x1_tt)
                msq1 = row_stats_sq(x1_tt, f"s2_{c}_{tt}")
                ln1 = colp.tile([128, 1], f32, name=f"ln1{c}_{tt}", tag="c1",
                                bufs=8)
                # mean(x1^2) = 16*msq1  (x1_tt = x1/4)
                nc.scalar.activation(ln1[:], msq1[:], AF.Ln, scale=16.0,
                                     bias=eps_e[:])
                s2big = colp.tile([128, 1], f32, name=f"s2b{c}_{tt}",
                                  tag="s2col", bufs=6)
                # 16 * exp(-ln1) compensates u2 being built from x1/4
                nc.scalar.activation(s2big[:], ln1[:], AF.Exp, scale=-1.0,
                                     bias=ln16_e[:])
                s2cols.append(s2big)

            # x1 feature-major (transposed) in 4 quarter tiles
            x1t_q = []
            for qi in range(4):
                t = wk.tile([128, 4, CH], bf, name=f"x1t{c}_{qi}", tag="x1t",
                            bufs=4)
                nc.sync.dma_start_transpose(
                    t[:], cout[csl, qi * 512:(qi + 1) * 512])
                x1t_q.append(t)

            # up-projection + relu^2
            u2s = []
            for f in range(FCT):
                wfc_f = wk.tile([128, EC, 128], bf, name=f"wfc{c}_{f}",
                                tag="we3", bufs=5)
                nc.sync.dma_start(
                    wfc_f[:],
                    io["wfcT"].rearrange("(a p) n -> p a n", p=128)
                    [:, :, f * 128:(f + 1) * 128])
                u_ps = psp.tile([128, CH], f32, name=f"ups{c}_{f}", tag="ps",
                                bufs=8)
                for e in range(EC):
                    nc.tensor.matmul(u_ps[:], wfc_f[:, e, :],
                                     x1t_q[e // 4][:, e % 4, :],
                                     start=(e == 0), stop=(e == EC - 1))
                ur = wk.tile([128, CH], bf, name=f"ur{c}_{f}", tag="p",
                             bufs=4)
                nc.scalar.activation(ur[:], u_ps[:], AF.Relu)
                u2 = wk.tile([128, CH], bf, name=f"u2{c}_{f}", tag="u2",
                             bufs=FCT + 1)
                nc.vector.tensor_tensor(u2[:], ur[:], ur[:], op=OP.mult)
                u2s.append(u2)

            # down-projection in 256-col E slices + residual partial
            for ot in range(8):
                osl = slice(ot * 256, (ot + 1) * 256)
                wm_ot = wk.tile([128, FCT, 256], bf, name=f"wm{c}_{ot}",
                                tag="wm", bufs=2)
                nc.sync.dma_start(
                    wm_ot[:],
                    io["wmlpT"].rearrange("(a p) n -> p a n", p=128)[:, :, osl])
                for tl in range(4):
                    tsl = slice(tl * 128, (tl + 1) * 128)
                    mp = psp.tile([128, 256], f32, name=f"mp{c}_{ot}_{tl}",
                                  tag="ps", bufs=8)
                    for f in range(FCT):
                        nc.tensor.matmul(mp[:], u2s[f][:, tsl], wm_ot[:, f, :],
                                         start=(f == 0), stop=(f == FCT - 1))
                    o_sb = wk.tile([128, 256], bf, name=f"o{c}_{ot}_{tl}",
                                   tag="of", bufs=3)
                    nc.vector.scalar_tensor_tensor(
                        o_sb[:], mp[:], s2cols[tl][:], x1_tts[tl][:, osl],
                        op0=OP.mult, op1=OP.add)
                    rows = slice(c * CH + tl * 128, c * CH + (tl + 1) * 128)
                    nc.sync.dma_start(io["out"][rows, osl], o_sb[:])


def _pin_act_tables():
    """Force every activation onto natural_log_exp_and_others (it contains
    Exp/Ln/Square/Relu/Copy/Identity) so the table is loaded once instead of
    thrashing between per-function sets. Indices are preserved; the kept
    set's real contents are unchanged, so runtime behavior is sound."""
    import concourse.bacc as bacc_mod
    import concourse.mybir as mybir
    if getattr(bacc_mod, "_act_tables_pinned", False):
        return
    AF = mybir.ActivationFunctionType
    mine = {AF.Exp, AF.Ln, AF.Square, AF.Relu, AF.Copy, AF.Identity}
    orig = bacc_mod.get_activation_tables

    def patched(arch):
        t = orig(arch)
        out = {}
        for name, funcs in t.items():
            if name == "natural_log_exp_and_others":
                out[name] = set(funcs)
            else:
                out[name] = set(funcs) - mine
        return out

    bacc_mod.get_activation_tables = patched
    bacc_mod._act_tables_pinned = True


def build_nc(T=T_FULL, num_devices=NCORES):
    from concourse import bacc
    import concourse.tile as tile
    _pin_act_tables()
    nc = bacc.Bacc("TRN2", target_bir_lowering=False, debug=False,
                   enable_asserts=True, num_devices=num_devices)
    io = declare_io(nc, T)
    with tile.TileContext(nc) as tc:
        emit(tc, io, T)
    nc.compile()
    return nc


def combine_outputs(results, T=T_FULL):
    out = np.zeros((B, T, E), np.float32)
    for c in range(NCORES):
        out[c // 4] += np.asarray(results[c]["out"]).astype(np.float32)
    return out


def kernel(**inputs):
    from concourse.bass_utils import run_bass_kernel_spmd
    in_maps = shard_inputs(**inputs)
    nc = build_nc(T_FULL)
    res = run_bass_kernel_spmd(nc, in_maps, core_ids=list(range(NCORES)))
    return combine_outputs(res.results, T_FULL)
